# revision 50
# baseline (speedup 1.0000x reference)
"""Sparse delta-V attention (GQA, non-causal) on 8 TRN2 NeuronCores.

Problem (S=2048, H=16, KVH=4, D=128, NS=1024 salient rows):
  v_delta      = v - v_cache[idx]
  v_cache_new  = v_cache.at[idx].set(v)
  o_salient    = attn(q[idx], k_rep, repeat(v_cache_new))        # full recompute
  new_c        = c_cache + attn(q, k_rep, repeat(scatter(v_delta)))
  new_c[idx]   = o_salient

Strategy:
  * Host applies a PERMUTATION (salient rows first) to q/k/v_cache/c_cache.
    Softmax over keys is permutation-invariant, so all sparse gathers and
    scatters become dense block slices.  Host also pre-transposes q, k and
    c_cache to [D, S] layout so the device does zero transposes.
  * Shard: 2 q-heads + their kv-head per core (tensor parallel over heads,
    GQA-aware).  No collectives; host re-assembles per-head outputs.
  * Matmul path runs in float16 (1 cycle/row on PE vs 4 for float32;
    11-bit mantissa keeps rel-err ~1e-3 for these O(1)-magnitude values).
  * Device per (head, q-group of 1024):
      - per k-tile: two N=512 matmuls fill one 2-bank PSUM tile; a single
        ACT exp over 1024 columns writes e^T [128k, 1024q] in f16
      - PV accumulates over k-tiles in PSUM: out^T += v_tile.T @ e^T
        (salient q-group g0 uses the updated cache over all 2048 keys,
        non-salient g1 uses v_delta over the 1024 salient keys only)
      - denominator: DVE f16 running sum of e tiles, then ones.T @ acc
      - normalize: reciprocal_approx_fast, PE-broadcast of 1/den (K=1
        matmul), DVE multiply, c_cache add for non-salient groups; the
        whole normalize chain for group i is emitted inside group i+1's
        tile loop so no engine stalls on it
  * PE warmup matmuls ramp the HAM clock gate during the input DMAs.
"""

import os
import sys

import numpy as np

sys.path.insert(0, "/opt/trn_rl_repo")

S = 2048
H = 16
KVH = 4
D = 128
NS = 1024
NCORES = 8
HPC = H // NCORES          # q heads per core
SCALE = 1.0 / float(np.sqrt(D))

QG = 1024                  # q columns per group (f16 moving operand: N<=1024)
NG = S // QG               # 2 q groups: g0 = salient q rows, g1 = non-salient
NT = S // 128              # 16 k tiles
NST = NS // 128            # 8 salient k tiles

TRACE = False
LAST_EXEC_NS = None
LAST_RESULTS = None
LDW_OPT = False  # --enable-ldw-opt=true crashes walrus codegen

_EPOOL = int(os.environ.get("K_EPOOL", "5"))
_OPOOL = int(os.environ.get("K_OPOOL", "2"))
_ACCP = int(os.environ.get("K_ACCP", "3"))
_WARM = int(os.environ.get("K_WARM", "10"))

_NC_CACHE = {}


def _patch_ldw_opt():
    """walrus is invoked with --enable-ldw-opt=false by default; LDW opt
    dedupes per-matmul LDWEIGHTS reloads, which dominate our PE overhead."""
    import concourse.bass_utils as bu

    if getattr(bu, "_ldw_patched", False):
        return
    orig = bu.run_command

    def patched(argv, **kw):
        argv = [
            a.replace("--enable-ldw-opt=false", "--enable-ldw-opt=true")
            if isinstance(a, str) else a
            for a in argv
        ]
        return orig(argv, **kw)

    bu.run_command = patched
    bu._ldw_patched = True


def _ensure_ntff_hook():
    """The agent image lacks ``antenv.axon_hooks``; synthesize it and
    register the ctypes NTFF profiling hook so trace=True works."""
    import types

    if "antenv.axon_hooks" in sys.modules:
        return
    mod = types.ModuleType("antenv.axon_hooks")
    holder = [None]
    mod.set_axon_ntff_profile_hook = lambda h: holder.__setitem__(0, h)
    mod.get_axon_ntff_profile_hook = lambda: holder[0]
    import antenv

    sys.modules["antenv.axon_hooks"] = mod
    antenv.axon_hooks = mod
    try:
        from trn_agent_boot.trn_boot import _ntff_profile_via_ctypes

        hook = _ntff_profile_via_ctypes("/opt/axon/libaxon_pjrt.so")
        if hook is not None:
            mod.set_axon_ntff_profile_hook(hook)
    except Exception:
        pass


def _build_nc():
    import concourse.mybir as mybir
    import concourse.tile as tile
    from concourse import bacc

    f32 = mybir.dt.float32
    f16 = mybir.dt.float16

    nc = bacc.Bacc(None, target_bir_lowering=False)

    qT = nc.declare_dram_parameter("qT", [HPC, D, S], f16, isOutput=False)
    kT = nc.declare_dram_parameter("kT", [D, S], f16, isOutput=False)
    vnew = nc.declare_dram_parameter("vnew", [S, D], f16, isOutput=False)
    vcs = nc.declare_dram_parameter("vcs", [NS, D], f16, isOutput=False)
    ccT = nc.declare_dram_parameter("ccT", [HPC, D, S - NS], f32, isOutput=False)
    out = nc.declare_dram_parameter("out", [HPC, D, S], f16, isOutput=True)

    EXP = mybir.ActivationFunctionType.Exp

    with tile.TileContext(nc) as tc:
        with (
            tc.tile_pool(name="big", bufs=1) as big,
            tc.tile_pool(name="epool", bufs=_EPOOL) as epool,
            tc.tile_pool(name="opool", bufs=_OPOOL) as opool,
            tc.tile_pool(name="accp", bufs=_ACCP) as accp,
            tc.tile_pool(name="ps", bufs=2, space="PSUM") as ps,
            tc.tile_pool(name="po", bufs=2, space="PSUM") as po,
        ):
            ones_sb = big.tile([128, 1], f16, tag="ones")
            nc.vector.memset(ones_sb, 1.0)
            # preload the ACT Exp table during the DMA wait (otherwise the
            # 1.3us table load lands in front of the first real exp)
            scratch_e = big.tile([128, 1], f16, tag="scr")
            nc.scalar.activation(scratch_e, ones_sb, EXP, scale=SCALE)
            # PE warmup: dummy matmuls with no DMA deps so the HAM clock
            # gate ramps toward 2.4 GHz while input DMAs land.
            warm_sb = big.tile([128, 640], f16, tag="warm")
            nc.gpsimd.memset(warm_sb, 0.0)
            psum_w = ps.tile([128, QG], f32, tag="ps")
            for i in range(_WARM):
                nc.tensor.matmul(
                    psum_w[:, :512],
                    warm_sb[:, :128],
                    warm_sb[:, 128:640],
                    start=True, stop=True, skip_group_check=True,
                )

            # --- inputs, chunked so the first-needed tiles land quickly
            kT_sb = big.tile([D, S], f16, tag="kT")
            qT_sb = big.tile([D, HPC * S], f16, tag="qT")
            vnew_sb = big.tile([128, NT * D], f16, tag="vnew")
            vcs_sb = big.tile([128, NST * D], f16, tag="vcs")
            ccT_sb = big.tile([D, HPC * (S - NS)], f32, tag="ccT")

            nc.sync.dma_start(kT_sb[:, :512], kT[:, :512])
            nc.sync.dma_start(qT_sb[:, :QG], qT[0][:, :QG])
            # v rows land as [k_within_tile (partition), tile*D + d]
            nc.sync.dma_start(
                vnew_sb[:, : NST * D].rearrange("p (t d) -> p t d", d=D),
                vnew[:].rearrange("(t p) d -> p t d", p=128)[:, :NST, :],
            )
            for c4 in range(1, 4):
                nc.sync.dma_start(
                    kT_sb[:, c4 * 512:(c4 + 1) * 512],
                    kT[:, c4 * 512:(c4 + 1) * 512],
                )
            nc.sync.dma_start(
                vnew_sb[:, NST * D:].rearrange("p (t d) -> p t d", d=D),
                vnew[:].rearrange("(t p) d -> p t d", p=128)[:, NST:, :],
            )
            for h in range(HPC):
                for g in range(NG):
                    if h == 0 and g == 0:
                        continue
                    nc.sync.dma_start(
                        qT_sb[:, h * S + g * QG: h * S + (g + 1) * QG],
                        qT[h][:, g * QG:(g + 1) * QG],
                    )
            nc.sync.dma_start(
                vcs_sb.rearrange("p (t d) -> p t d", d=D),
                vcs[:].rearrange("(t p) d -> p t d", p=128),
            )
            for h in range(HPC):
                nc.sync.dma_start(
                    ccT_sb[:, h * (S - NS):(h + 1) * (S - NS)], ccT[h]
                )

            vd_sb = big.tile([128, NST * D], f16, tag="vd")
            nc.vector.tensor_sub(vd_sb, vnew_sb[:, : NST * D], vcs_sb)

            HQ = QG // 2  # 512: max matmul N (one PSUM bank)

            onesr_sb = big.tile([1, 128], f16, tag="onesr")
            nc.vector.memset(onesr_sb, 1.0)

            def emit_norm_a1(st):
                """Deferred normalize stage A1: denominator matmuls."""
                h, g, salient_g, psum_o, acc = st
                psum_d = ps.tile([128, QG], f32, tag="ps")
                for u in range(2):
                    nc.tensor.matmul(
                        psum_d[0:1, u * HQ:(u + 1) * HQ],
                        ones_sb,
                        acc[:, u * HQ:(u + 1) * HQ],
                        start=True, stop=True, skip_group_check=True,
                    )
                return st + (psum_d,)

            def emit_norm_a2(st):
                """Deferred normalize stage A2: DVE reciprocal + f16 cast."""
                h, g, salient_g, psum_o, acc, psum_d = st
                r_sb = opool.tile([1, QG], f32, tag="r")
                nc.vector.reciprocal_approx_fast(out=r_sb, in_=psum_d[0:1, :])
                r16 = opool.tile([1, QG], f16, tag="r16")
                nc.vector.tensor_copy(r16, r_sb)
                return st[:5] + (r16,)

            def emit_norm_a(st):
                return emit_norm_a2(emit_norm_a1(st))

            def emit_norm_b(st, last=False):
                """Deferred normalize stage B: PE broadcast of 1/den, DVE
                multiply (+ c_cache add), output DMA.  For the final group
                the whole chain is on the critical path, so process it in
                512-column halves to pipeline DVE ops with PE and DMA."""
                h, g, salient_g, psum_o, acc, r16 = st
                if last:
                    psum_b = ps.tile([128, QG], f32, tag="ps")
                    b_sb = opool.tile([128, QG], f32, tag="b")
                    o_sb = opool.tile([128, QG], f16, tag="o")
                    for u in range(2):
                        sl = slice(u * HQ, (u + 1) * HQ)
                        nc.tensor.matmul(
                            psum_b[:, sl], onesr_sb, r16[:, sl],
                            start=True, stop=True, skip_group_check=True,
                        )
                        nc.vector.tensor_copy(b_sb[:, sl], psum_b[:, sl])
                        nc.vector.tensor_mul(
                            o_sb[:, sl], psum_o[:, sl], b_sb[:, sl]
                        )
                        if not salient_g:
                            cc = ccT_sb[:, h * (S - NS) + u * HQ:
                                        h * (S - NS) + (u + 1) * HQ]
                            nc.vector.tensor_add(o_sb[:, sl], o_sb[:, sl], cc)
                        nc.sync.dma_start(
                            out[h][:, g * QG + u * HQ: g * QG + (u + 1) * HQ],
                            o_sb[:, sl],
                        )
                    return
                psum_b = ps.tile([128, QG], f32, tag="ps")
                for u in range(2):
                    nc.tensor.matmul(
                        psum_b[:, u * HQ:(u + 1) * HQ],
                        onesr_sb,
                        r16[:, u * HQ:(u + 1) * HQ],
                        start=True, stop=True, skip_group_check=True,
                    )
                b_sb = opool.tile([128, QG], f32, tag="b")
                nc.vector.tensor_copy(b_sb, psum_b)
                o_sb = opool.tile([128, QG], f16, tag="o")
                nc.vector.tensor_mul(o_sb, psum_o, b_sb)
                if not salient_g:
                    cc = ccT_sb[:, h * (S - NS):(h + 1) * (S - NS)]
                    nc.vector.tensor_add(o_sb, o_sb, cc)
                nc.sync.dma_start(out[h][:, g * QG:(g + 1) * QG], o_sb)

            pending = None
            first_group = True
            for h in range(HPC):
                for g in ((0, 1) if h == 0 else (1, 0)):
                    salient_g = g == 0
                    pv_tiles = NT if salient_g else NST
                    w_sb = vnew_sb if salient_g else vd_sb
                    psum_o = po.tile([128, QG], f32, tag="po")
                    acc = accp.tile([128, QG], f16, tag="acc")
                    q0 = h * S + g * QG
                    # host the previous group's normalize in PV-free tiles
                    # when this group has them (t>=NST), else early tiles
                    na_t, nb_t = (1, 4) if salient_g else (NST + 1, NST + 3)
                    # first group: run 4 scores/exp tiles before any PV so
                    # PE isn't stalled behind the v-cache DMA
                    defer_pv = 4 if first_group else 0
                    first_group = False
                    deferred = []

                    def emit_pv(t):
                        for u in range(2):
                            nc.tensor.matmul(
                                psum_o[:, u * HQ:(u + 1) * HQ],
                                w_sb[:, t * D:(t + 1) * D],
                                e_tiles[t][:, u * HQ:(u + 1) * HQ],
                                start=(t == 0), stop=(t == pv_tiles - 1),
                                skip_group_check=True,
                            )

                    e_tiles = {}
                    for t in range(NT):
                        # two N=512 matmuls fill one 2-bank psum tile; a
                        # single exp over 1024 columns amortizes ACT latency
                        psum_s = ps.tile([128, QG], f32, tag="ps")
                        for u in range(2):
                            nc.tensor.matmul(
                                psum_s[:, u * HQ:(u + 1) * HQ],
                                kT_sb[:, t * 128:(t + 1) * 128],
                                qT_sb[:, q0 + u * HQ: q0 + (u + 1) * HQ],
                                start=True, stop=True, skip_group_check=True,
                            )
                        e_t = epool.tile([128, QG], f16, tag="e")
                        nc.scalar.activation(e_t, psum_s, EXP, scale=SCALE)
                        e_tiles[t] = e_t
                        if t < pv_tiles:
                            if t < defer_pv:
                                deferred.append(t)
                            else:
                                for dt_ in deferred:
                                    emit_pv(dt_)
                                deferred = []
                                emit_pv(t)
                        # running f16 sum of e tiles on DVE (replaces the
                        # per-tile ones-matmul denominator on PE)
                        if t == 0:
                            nc.vector.tensor_copy(acc, e_t)
                        else:
                            nc.vector.tensor_add(acc, acc, e_t)
                        if t == na_t and pending is not None:
                            pending = emit_norm_a(pending)
                        if t == nb_t and pending is not None:
                            emit_norm_b(pending)
                            pending = None
                    for dt_ in deferred:
                        emit_pv(dt_)
                    pending = (h, g, salient_g, psum_o, acc)
            emit_norm_b(emit_norm_a(pending), last=True)
    nc.finalize()
    return nc


def _get_nc():
    if "nc" not in _NC_CACHE:
        _NC_CACHE["nc"] = _build_nc()
    return _NC_CACHE["nc"]


def kernel(**inputs) -> np.ndarray:
    global LAST_EXEC_NS, LAST_RESULTS
    from concourse.bass_utils import run_bass_kernel_spmd

    q = np.ascontiguousarray(np.asarray(inputs["q"], dtype=np.float32))
    k = np.ascontiguousarray(np.asarray(inputs["k"], dtype=np.float32))
    v = np.ascontiguousarray(np.asarray(inputs["v"], dtype=np.float32))
    v_cache = np.ascontiguousarray(np.asarray(inputs["v_cache"], dtype=np.float32))
    c_cache = np.ascontiguousarray(np.asarray(inputs["c_cache"], dtype=np.float32))
    idx = np.asarray(inputs["idx_salient"]).astype(np.int64)

    mask = np.zeros(S, dtype=bool)
    mask[idx] = True
    nonsal = np.nonzero(~mask)[0]
    perm = np.concatenate([idx, nonsal])

    qp = q[perm].astype(np.float16)
    kp = k[perm].astype(np.float16)
    ccp = c_cache[perm]

    in_maps = []
    for c in range(NCORES):
        kvh = (HPC * c) // (H // KVH)
        hs = list(range(HPC * c, HPC * (c + 1)))
        qT = np.ascontiguousarray(qp[:, hs, :].transpose(1, 2, 0))
        kT = np.ascontiguousarray(kp[:, kvh, :].T)
        vnew = np.ascontiguousarray(
            np.concatenate(
                [v[:, kvh, :], v_cache[nonsal, kvh, :]], axis=0
            ).astype(np.float16)
        )
        vcs = np.ascontiguousarray(v_cache[idx, kvh, :].astype(np.float16))
        ccT = np.ascontiguousarray(ccp[NS:, hs, :].transpose(1, 2, 0))
        in_maps.append({"qT": qT, "kT": kT, "vnew": vnew, "vcs": vcs, "ccT": ccT})

    nc = _get_nc()
    if LDW_OPT:
        _patch_ldw_opt()
    if TRACE:
        _ensure_ntff_hook()
    res = run_bass_kernel_spmd(
        nc, in_maps, core_ids=list(range(NCORES)), trace=TRACE
    )
    LAST_EXEC_NS = res.exec_time_ns
    LAST_RESULTS = res

    outp = np.empty((S, H, D), dtype=np.float32)
    for c in range(NCORES):
        o = res.results[c]["out"]
        for j in range(HPC):
            outp[:, HPC * c + j, :] = o[j].T
    full = np.empty_like(outp)
    full[perm] = outp
    return full



# revision 51
# speedup vs baseline: 1.0078x; 1.0078x over previous
"""Sparse delta-V attention (GQA, non-causal) on 8 TRN2 NeuronCores.

Problem (S=2048, H=16, KVH=4, D=128, NS=1024 salient rows):
  v_delta      = v - v_cache[idx]
  v_cache_new  = v_cache.at[idx].set(v)
  o_salient    = attn(q[idx], k_rep, repeat(v_cache_new))        # full recompute
  new_c        = c_cache + attn(q, k_rep, repeat(scatter(v_delta)))
  new_c[idx]   = o_salient

Strategy:
  * Host applies a PERMUTATION (salient rows first) to q/k/v_cache/c_cache.
    Softmax over keys is permutation-invariant, so all sparse gathers and
    scatters become dense block slices.  Host also pre-transposes q, k and
    c_cache to [D, S] layout so the device does zero transposes.
  * Shard: 2 q-heads + their kv-head per core (tensor parallel over heads,
    GQA-aware).  No collectives; host re-assembles per-head outputs.
  * Matmul path runs in float16 (1 cycle/row on PE vs 4 for float32;
    11-bit mantissa keeps rel-err ~1e-3 for these O(1)-magnitude values).
  * Device per (head, q-group of 1024):
      - per k-tile: two N=512 matmuls fill one 2-bank PSUM tile; a single
        ACT exp over 1024 columns writes e^T [128k, 1024q] in f16
      - PV accumulates over k-tiles in PSUM: out^T += v_tile.T @ e^T
        (salient q-group g0 uses the updated cache over all 2048 keys,
        non-salient g1 uses v_delta over the 1024 salient keys only)
      - denominator: DVE f16 running sum of e tiles, then ones.T @ acc
      - normalize: reciprocal_approx_fast, PE-broadcast of 1/den (K=1
        matmul), DVE multiply, c_cache add for non-salient groups; the
        whole normalize chain for group i is emitted inside group i+1's
        tile loop so no engine stalls on it
  * PE warmup matmuls ramp the HAM clock gate during the input DMAs.
"""

import os
import sys

import numpy as np

sys.path.insert(0, "/opt/trn_rl_repo")

S = 2048
H = 16
KVH = 4
D = 128
NS = 1024
NCORES = 8
HPC = H // NCORES          # q heads per core
SCALE = 1.0 / float(np.sqrt(D))

QG = 1024                  # q columns per group (f16 moving operand: N<=1024)
NG = S // QG               # 2 q groups: g0 = salient q rows, g1 = non-salient
NT = S // 128              # 16 k tiles
NST = NS // 128            # 8 salient k tiles

TRACE = False
LAST_EXEC_NS = None
LAST_RESULTS = None
LDW_OPT = False  # --enable-ldw-opt=true crashes walrus codegen

_EPOOL = int(os.environ.get("K_EPOOL", "5"))
_OPOOL = int(os.environ.get("K_OPOOL", "2"))
_ACCP = int(os.environ.get("K_ACCP", "3"))
_WARM = int(os.environ.get("K_WARM", "10"))

_NC_CACHE = {}


def _patch_ldw_opt():
    """walrus is invoked with --enable-ldw-opt=false by default; LDW opt
    dedupes per-matmul LDWEIGHTS reloads, which dominate our PE overhead."""
    import concourse.bass_utils as bu

    if getattr(bu, "_ldw_patched", False):
        return
    orig = bu.run_command

    def patched(argv, **kw):
        argv = [
            a.replace("--enable-ldw-opt=false", "--enable-ldw-opt=true")
            if isinstance(a, str) else a
            for a in argv
        ]
        return orig(argv, **kw)

    bu.run_command = patched
    bu._ldw_patched = True


def _ensure_ntff_hook():
    """The agent image lacks ``antenv.axon_hooks``; synthesize it and
    register the ctypes NTFF profiling hook so trace=True works."""
    import types

    if "antenv.axon_hooks" in sys.modules:
        return
    mod = types.ModuleType("antenv.axon_hooks")
    holder = [None]
    mod.set_axon_ntff_profile_hook = lambda h: holder.__setitem__(0, h)
    mod.get_axon_ntff_profile_hook = lambda: holder[0]
    import antenv

    sys.modules["antenv.axon_hooks"] = mod
    antenv.axon_hooks = mod
    try:
        from trn_agent_boot.trn_boot import _ntff_profile_via_ctypes

        hook = _ntff_profile_via_ctypes("/opt/axon/libaxon_pjrt.so")
        if hook is not None:
            mod.set_axon_ntff_profile_hook(hook)
    except Exception:
        pass


def _build_nc():
    import concourse.mybir as mybir
    import concourse.tile as tile
    from concourse import bacc

    f32 = mybir.dt.float32
    f16 = mybir.dt.float16

    nc = bacc.Bacc(None, target_bir_lowering=False)

    qT = nc.declare_dram_parameter("qT", [HPC, D, S], f16, isOutput=False)
    kT = nc.declare_dram_parameter("kT", [D, S], f16, isOutput=False)
    vnew = nc.declare_dram_parameter("vnew", [S, D], f16, isOutput=False)
    vcs = nc.declare_dram_parameter("vcs", [NS, D], f16, isOutput=False)
    ccT = nc.declare_dram_parameter("ccT", [HPC, D, S - NS], f32, isOutput=False)
    out = nc.declare_dram_parameter("out", [HPC, D, S], f16, isOutput=True)

    EXP = mybir.ActivationFunctionType.Exp

    with tile.TileContext(nc) as tc:
        with (
            tc.tile_pool(name="big", bufs=1) as big,
            tc.tile_pool(name="epool", bufs=_EPOOL) as epool,
            tc.tile_pool(name="opool", bufs=_OPOOL) as opool,
            tc.tile_pool(name="accp", bufs=_ACCP) as accp,
            tc.tile_pool(name="ps", bufs=2, space="PSUM") as ps,
            tc.tile_pool(name="po", bufs=2, space="PSUM") as po,
        ):
            ones_sb = big.tile([128, 1], f16, tag="ones")
            nc.vector.memset(ones_sb, 1.0)
            # preload the ACT Exp table during the DMA wait (otherwise the
            # 1.3us table load lands in front of the first real exp)
            scratch_e = big.tile([128, 1], f16, tag="scr")
            nc.scalar.activation(scratch_e, ones_sb, EXP, scale=SCALE)
            # PE warmup: dummy matmuls with no DMA deps so the HAM clock
            # gate ramps toward 2.4 GHz while input DMAs land.
            warm_sb = big.tile([128, 640], f16, tag="warm")
            nc.gpsimd.memset(warm_sb, 0.0)
            psum_w = ps.tile([128, QG], f32, tag="ps")
            for i in range(_WARM):
                nc.tensor.matmul(
                    psum_w[:, :512],
                    warm_sb[:, :128],
                    warm_sb[:, 128:640],
                    start=True, stop=True, skip_group_check=True,
                )

            # --- inputs, chunked so the first-needed tiles land quickly
            kT_sb = big.tile([D, S], f16, tag="kT")
            qT_sb = big.tile([D, HPC * S], f16, tag="qT")
            vnew_sb = big.tile([128, NT * D], f16, tag="vnew")
            vcs_sb = big.tile([128, NST * D], f16, tag="vcs")
            ccT_sb = big.tile([D, HPC * (S - NS)], f32, tag="ccT")

            nc.sync.dma_start(kT_sb[:, :512], kT[:, :512])
            nc.sync.dma_start(qT_sb[:, :QG], qT[0][:, :QG])
            # v rows land as [k_within_tile (partition), tile*D + d]
            nc.sync.dma_start(
                vnew_sb[:, : NST * D].rearrange("p (t d) -> p t d", d=D),
                vnew[:].rearrange("(t p) d -> p t d", p=128)[:, :NST, :],
            )
            for c4 in range(1, 4):
                nc.sync.dma_start(
                    kT_sb[:, c4 * 512:(c4 + 1) * 512],
                    kT[:, c4 * 512:(c4 + 1) * 512],
                )
            nc.sync.dma_start(
                vnew_sb[:, NST * D:].rearrange("p (t d) -> p t d", d=D),
                vnew[:].rearrange("(t p) d -> p t d", p=128)[:, NST:, :],
            )
            for h in range(HPC):
                for g in range(NG):
                    if h == 0 and g == 0:
                        continue
                    nc.sync.dma_start(
                        qT_sb[:, h * S + g * QG: h * S + (g + 1) * QG],
                        qT[h][:, g * QG:(g + 1) * QG],
                    )
            nc.sync.dma_start(
                vcs_sb.rearrange("p (t d) -> p t d", d=D),
                vcs[:].rearrange("(t p) d -> p t d", p=128),
            )
            for h in range(HPC):
                nc.sync.dma_start(
                    ccT_sb[:, h * (S - NS):(h + 1) * (S - NS)], ccT[h]
                )

            vd_sb = big.tile([128, NST * D], f16, tag="vd")
            nc.vector.tensor_sub(vd_sb, vnew_sb[:, : NST * D], vcs_sb)

            HQ = QG // 2  # 512: max matmul N (one PSUM bank)

            onesr_sb = big.tile([1, 128], f16, tag="onesr")
            nc.vector.memset(onesr_sb, 1.0)

            def emit_norm_a1(st):
                """Deferred normalize stage A1: denominator matmuls."""
                h, g, salient_g, psum_o, acc = st
                psum_d = ps.tile([128, QG], f32, tag="ps")
                for u in range(2):
                    nc.tensor.matmul(
                        psum_d[0:1, u * HQ:(u + 1) * HQ],
                        ones_sb,
                        acc[:, u * HQ:(u + 1) * HQ],
                        start=True, stop=True, skip_group_check=True,
                    )
                return st + (psum_d,)

            def emit_norm_a2(st):
                """Deferred normalize stage A2: DVE reciprocal + f16 cast."""
                h, g, salient_g, psum_o, acc, psum_d = st
                r_sb = opool.tile([1, QG], f32, tag="r")
                nc.vector.reciprocal_approx_fast(out=r_sb, in_=psum_d[0:1, :])
                r16 = opool.tile([1, QG], f16, tag="r16")
                nc.vector.tensor_copy(r16, r_sb)
                return st[:5] + (r16,)

            def emit_norm_a(st):
                return emit_norm_a2(emit_norm_a1(st))

            def emit_norm_b(st, last=False):
                """Deferred normalize stage B: PE broadcast of 1/den, DVE
                multiply (+ c_cache add), output DMA.  For the final group
                the whole chain is on the critical path, so process it in
                512-column halves to pipeline DVE ops with PE and DMA."""
                h, g, salient_g, psum_o, acc, r16 = st
                if last:
                    psum_b = ps.tile([128, QG], f32, tag="ps")
                    b_sb = opool.tile([128, QG], f32, tag="b")
                    o_sb = opool.tile([128, QG], f16, tag="o")
                    for u in range(2):
                        sl = slice(u * HQ, (u + 1) * HQ)
                        nc.tensor.matmul(
                            psum_b[:, sl], onesr_sb, r16[:, sl],
                            start=True, stop=True, skip_group_check=True,
                        )
                        nc.vector.tensor_copy(b_sb[:, sl], psum_b[:, sl])
                        nc.vector.tensor_mul(
                            o_sb[:, sl], psum_o[:, sl], b_sb[:, sl]
                        )
                        if not salient_g:
                            cc = ccT_sb[:, h * (S - NS) + u * HQ:
                                        h * (S - NS) + (u + 1) * HQ]
                            nc.vector.tensor_add(o_sb[:, sl], o_sb[:, sl], cc)
                        nc.sync.dma_start(
                            out[h][:, g * QG + u * HQ: g * QG + (u + 1) * HQ],
                            o_sb[:, sl],
                        )
                    return
                psum_b = ps.tile([128, QG], f32, tag="ps")
                for u in range(2):
                    nc.tensor.matmul(
                        psum_b[:, u * HQ:(u + 1) * HQ],
                        onesr_sb,
                        r16[:, u * HQ:(u + 1) * HQ],
                        start=True, stop=True, skip_group_check=True,
                    )
                b_sb = opool.tile([128, QG], f32, tag="b")
                nc.vector.tensor_copy(b_sb, psum_b)
                o_sb = opool.tile([128, QG], f16, tag="o")
                nc.vector.tensor_mul(o_sb, psum_o, b_sb)
                if not salient_g:
                    cc = ccT_sb[:, h * (S - NS):(h + 1) * (S - NS)]
                    nc.vector.tensor_add(o_sb, o_sb, cc)
                nc.sync.dma_start(out[h][:, g * QG:(g + 1) * QG], o_sb)

            pending = None
            first_group = True
            for h in range(HPC):
                for g in ((0, 1) if h == 0 else (1, 0)):
                    salient_g = g == 0
                    pv_tiles = NT if salient_g else NST
                    w_sb = vnew_sb if salient_g else vd_sb
                    psum_o = po.tile([128, QG], f32, tag="po")
                    acc = accp.tile([128, QG], f16, tag="acc")
                    q0 = h * S + g * QG
                    # host the previous group's normalize in PV-free tiles
                    # when this group has them (t>=NST), else early tiles
                    na_t, nb_t = (2, 6) if salient_g else (NST + 1, NST + 4)
                    # first group: run 4 scores/exp tiles before any PV so
                    # PE isn't stalled behind the v-cache DMA
                    defer_pv = 4 if first_group else 0
                    first_group = False
                    deferred = []

                    def emit_pv(t):
                        for u in range(2):
                            nc.tensor.matmul(
                                psum_o[:, u * HQ:(u + 1) * HQ],
                                w_sb[:, t * D:(t + 1) * D],
                                e_tiles[t][:, u * HQ:(u + 1) * HQ],
                                start=(t == 0), stop=(t == pv_tiles - 1),
                                skip_group_check=True,
                            )

                    e_tiles = {}
                    for t in range(NT):
                        # two N=512 matmuls fill one 2-bank psum tile; a
                        # single exp over 1024 columns amortizes ACT latency
                        psum_s = ps.tile([128, QG], f32, tag="ps")
                        for u in range(2):
                            nc.tensor.matmul(
                                psum_s[:, u * HQ:(u + 1) * HQ],
                                kT_sb[:, t * 128:(t + 1) * 128],
                                qT_sb[:, q0 + u * HQ: q0 + (u + 1) * HQ],
                                start=True, stop=True, skip_group_check=True,
                            )
                        e_t = epool.tile([128, QG], f16, tag="e")
                        nc.scalar.activation(e_t, psum_s, EXP, scale=SCALE)
                        e_tiles[t] = e_t
                        if t < pv_tiles:
                            if t < defer_pv:
                                deferred.append(t)
                            else:
                                for dt_ in deferred:
                                    emit_pv(dt_)
                                deferred = []
                                emit_pv(t)
                        # running f16 sum of e tiles on DVE (replaces the
                        # per-tile ones-matmul denominator on PE)
                        if t == 0:
                            nc.vector.tensor_copy(acc, e_t)
                        else:
                            nc.vector.tensor_add(acc, acc, e_t)
                        if t == na_t and pending is not None:
                            pending = emit_norm_a(pending)
                        if t == nb_t and pending is not None:
                            emit_norm_b(pending)
                            pending = None
                    for dt_ in deferred:
                        emit_pv(dt_)
                    pending = (h, g, salient_g, psum_o, acc)
            emit_norm_b(emit_norm_a(pending), last=True)
    nc.finalize()
    return nc


def _get_nc():
    if "nc" not in _NC_CACHE:
        _NC_CACHE["nc"] = _build_nc()
    return _NC_CACHE["nc"]


def kernel(**inputs) -> np.ndarray:
    global LAST_EXEC_NS, LAST_RESULTS
    from concourse.bass_utils import run_bass_kernel_spmd

    q = np.ascontiguousarray(np.asarray(inputs["q"], dtype=np.float32))
    k = np.ascontiguousarray(np.asarray(inputs["k"], dtype=np.float32))
    v = np.ascontiguousarray(np.asarray(inputs["v"], dtype=np.float32))
    v_cache = np.ascontiguousarray(np.asarray(inputs["v_cache"], dtype=np.float32))
    c_cache = np.ascontiguousarray(np.asarray(inputs["c_cache"], dtype=np.float32))
    idx = np.asarray(inputs["idx_salient"]).astype(np.int64)

    mask = np.zeros(S, dtype=bool)
    mask[idx] = True
    nonsal = np.nonzero(~mask)[0]
    perm = np.concatenate([idx, nonsal])

    qp = q[perm].astype(np.float16)
    kp = k[perm].astype(np.float16)
    ccp = c_cache[perm]

    in_maps = []
    for c in range(NCORES):
        kvh = (HPC * c) // (H // KVH)
        hs = list(range(HPC * c, HPC * (c + 1)))
        qT = np.ascontiguousarray(qp[:, hs, :].transpose(1, 2, 0))
        kT = np.ascontiguousarray(kp[:, kvh, :].T)
        vnew = np.ascontiguousarray(
            np.concatenate(
                [v[:, kvh, :], v_cache[nonsal, kvh, :]], axis=0
            ).astype(np.float16)
        )
        vcs = np.ascontiguousarray(v_cache[idx, kvh, :].astype(np.float16))
        ccT = np.ascontiguousarray(ccp[NS:, hs, :].transpose(1, 2, 0))
        in_maps.append({"qT": qT, "kT": kT, "vnew": vnew, "vcs": vcs, "ccT": ccT})

    nc = _get_nc()
    if LDW_OPT:
        _patch_ldw_opt()
    if TRACE:
        _ensure_ntff_hook()
    res = run_bass_kernel_spmd(
        nc, in_maps, core_ids=list(range(NCORES)), trace=TRACE
    )
    LAST_EXEC_NS = res.exec_time_ns
    LAST_RESULTS = res

    outp = np.empty((S, H, D), dtype=np.float32)
    for c in range(NCORES):
        o = res.results[c]["out"]
        for j in range(HPC):
            outp[:, HPC * c + j, :] = o[j].T
    full = np.empty_like(outp)
    full[perm] = outp
    return full

